# revision 30
# baseline (speedup 1.0000x reference)
"""Trainium2 Bass kernel for nn_MeanStdStiffRegularizer (segment reduce).

Strategy (8 NeuronCores, segment-sharded, sort-based):
  - The host shards BY SEGMENT: core c owns segments [64c, 64c+64).  Edges
    are permuted (stable sort by (segment, sign)) and laid out so that each
    (segment, sign) group occupies a fixed [128, 128] block of the per-core
    [128, 16384] fp8e5m2 image; column f = t*128 + (2*seg_local + sign).
    Only |x| is shipped (sign is encoded in the column parity), so no idx
    tensor and no abs op on device.  e5m2 keeps log-domain error ~1e-3 on
    the final loss (e4m3's subnormal floor would not).  Pads are 1.0
    (ln(1+eps) ~ 0) and group-capacity overflow (~0.3% of edges at the
    mean-sized capacity) is compensated exactly on the host in float64.
  - Device per core: Ln(|x|+eps) on ScalarE (the 1 elem/lane/cycle floor,
    chunked to pipeline behind the DMAs), L^2 on VectorE (bf16
    tensor_tensor 2x), and every segment sum on the PE: matmul with a
    ones[128,1] stationary against 512-wide moving slabs accumulated in
    PSUM.  Four PE column strips (tile_position) run concurrently; PSUM
    column j accumulates group g = j%128.  The x-stream matmuls are
    emitted first (they only depend on DMAs) so their accumulators drain
    mid-kernel, off the critical tail.
  - No collective: each core returns 3x4x[512] f32 strip partials; the
    host folds strips/replicas and does the final 512-sized math in f64.
"""

import sys
import types

import numpy as np

N_EDGES = 16777216
NUM_SEG = 512
STRENGTH = 0.01
STD_WEIGHT = 0.5
EPS = 1e-6

N_CORES = 8
P = 128
SEG_PER_CORE = NUM_SEG // N_CORES  # 64
N_GRP = 2 * SEG_PER_CORE  # 128 (seg, sign) groups per core
TPP = 128  # elems per partition per (seg, sign) group
C2 = P * TPP  # 16384 capacity per (seg, sign) group (~0.3% spill to host)
F_TOT = N_GRP * TPP  # 16384 free elems per partition
SLAB = 512
N_STRIP = 4
# DMA chunk boundaries (slab-aligned; small first chunks for fast ramp)
CHUNKS = (512, 1024, 2048, 4096, 4096, 2048, 1024, 1024, 512)
assert sum(CHUNKS) == F_TOT
# ACT/TT chunk boundaries: finer than DMA up to the last delivery-gated
# chunk (col 7680), then merged — those DMA sems fire well before the
# saturated Ln chain arrives, so fewer instructions = less overhead on the
# critical path.  Last chunk stays small for a short tail cascade.
ACT_CHUNKS = (512, 1024, 2048, 4096, 6144, 2048, 512)
assert sum(ACT_CHUNKS) == F_TOT


def _install_ntff_hook():
    """Register the axon NTFF profiling hook (missing antenv.axon_hooks)."""
    if "antenv.axon_hooks" in sys.modules:
        return
    mod = types.ModuleType("antenv.axon_hooks")
    _h = [None]
    mod.set_axon_ntff_profile_hook = lambda h: _h.__setitem__(0, h)
    mod.get_axon_ntff_profile_hook = lambda: _h[0]
    sys.modules["antenv.axon_hooks"] = mod
    try:
        from trn_agent_boot.trn_boot import _ntff_profile_via_ctypes

        mod.set_axon_ntff_profile_hook(
            _ntff_profile_via_ctypes("/opt/axon/libaxon_pjrt.so")
        )
    except Exception:
        pass


_NO_SPLIT_OPCODES = {
    "CollectiveCompute",
}


def _split_sync_waits(bir_json_bytes):
    """Rewrite BIR so no TPB instruction carries more than one sync wait.

    The walrus codegen in this container supports a single sync-wait slot
    per TPB instruction ("Too many sync wait commands" otherwise).  Extra
    waits are hoisted onto EventSemaphore instructions inserted immediately
    before, on the same engine (same issue-gating semantics).
    """
    import json

    j = json.loads(bir_json_bytes)
    n_split = 0
    uid = [0]
    for f in j["functions"]:
        for b in f["blocks"]:
            out = []
            for ins in b["instructions"]:
                si = ins.get("sync_info")
                ow = (si or {}).get("on_wait") or []
                if len(ow) > 1 and ins.get("opcode") not in _NO_SPLIT_OPCODES:
                    for w in ow[:-1]:
                        uid[0] += 1
                        out.append(
                            {
                                "debug": ins.get("debug", 0),
                                "engine": ins["engine"],
                                "ins": [],
                                "name": f"{ins['name']}-wsplit{uid[0]}",
                                "opcode": "EventSemaphore",
                                "outs": [],
                                "sync_info": {"on_update": [], "on_wait": [w]},
                            }
                        )
                    si["on_wait"] = [ow[-1]]
                    n_split += 1
                out.append(ins)
            b["instructions"] = out
    return json.dumps(j).encode(), n_split


def build_nc(n_cores=N_CORES):
    """Build the per-core Bass program (SPMD: same program on every core)."""
    import concourse.bass as bass
    import concourse.tile as tile
    from concourse import mybir

    f32 = mybir.dt.float32
    bf16 = mybir.dt.bfloat16
    AOP = mybir.AluOpType
    ACT = mybir.ActivationFunctionType

    nc = bass.Bass(
        "TRN2", target_bir_lowering=False, debug=False, num_devices=n_cores
    )
    f8 = mybir.dt.float8e5
    xs_ds = [
        nc.dram_tensor(f"xs{ci}", [P, fm], f8, kind="ExternalInput")
        for ci, fm in enumerate(CHUNKS)
    ]
    out_d = nc.dram_tensor(
        "out", [N_STRIP, 3 * SLAB], f32, kind="ExternalOutput"
    )

    with tile.TileContext(nc) as tc:
        with (
            tc.tile_pool(name="const", bufs=1) as cpool,
            tc.tile_pool(name="big", bufs=1) as big,
            tc.tile_pool(name="fin", bufs=1) as fin,
            tc.tile_pool(name="acc", bufs=1, space="PSUM") as psum,
        ):
            ones = cpool.tile([P, 1], bf16)
            nc.vector.memset(ones[:], 1.0)
            ones8 = cpool.tile([P, 1], f8)
            nc.vector.memset(ones8[:], 1.0)
            eps_t = cpool.tile([P, 1], f32)
            nc.vector.memset(eps_t[:], EPS)

            # single resident tiles; DMA/compute operate on column regions
            xt = big.tile([P, F_TOT], f8, name="xt")
            lx = big.tile([P, F_TOT], bf16, name="lx")
            sq = big.tile([P, F_TOT], bf16, name="sq")

            accs = [
                psum.tile([P, SLAB], f32, tag=f"acc{s}", name=f"acc{s}")
                for s in range(3)
            ]

            # MM schedule: all x-stream matmuls first (they depend only on
            # the DMAs, so the x accumulators close early), then per chunk
            # lx-mms followed by sq-mms.  Strip of the i-th emitted matmul
            # = i % N_STRIP so consecutive PE instructions always hit
            # different column strips (max concurrency, incl. the tail).
            slab_bounds = []
            f0 = 0
            for fm in CHUNKS:
                for j in range(fm // SLAB):
                    slab_bounds.append(f0 + j * SLAB)
                f0 += fm
            n_slab_total = F_TOT // SLAB
            sched = [(0, b) for b in slab_bounds]  # x block
            f0 = 0
            for fm in ACT_CHUNKS:
                for s in (1, 2):
                    for j in range(fm // SLAB):
                        sched.append((s, f0 + j * SLAB))
                f0 += fm
            group_total = [[0] * N_STRIP for _ in range(3)]
            for i, (s, _b) in enumerate(sched):
                group_total[s][i % N_STRIP] += 1

            # input DMAs in order: chunk 0 from Scalar (earliest preamble),
            # the rest from Sync (in-order transfers keep early chunks from
            # being starved by round-robin with later big ones)
            f0 = 0
            for ci, fm in enumerate(CHUNKS):
                cs = slice(f0, f0 + fm)
                eng = nc.scalar if ci == 0 else nc.sync
                eng.dma_start(xt[:, cs], xs_ds[ci][:])
                f0 += fm

            outsb = fin.tile([P, 3 * SLAB], f32)

            nmm = [[0] * N_STRIP for _ in range(3)]
            mm_idx = 0

            def emit_mm(s, b):
                nonlocal mm_idx
                src = (xt, lx, sq)[s]
                q = mm_idx % N_STRIP
                nc.tensor.matmul(
                    accs[s][32 * q : 32 * q + 1, :],
                    ones8[:] if s == 0 else ones[:],
                    src[:, b : b + SLAB],
                    start=(nmm[s][q] == 0),
                    stop=(nmm[s][q] == group_total[s][q] - 1),
                    tile_position=(0, 32 * q),
                )
                nmm[s][q] += 1
                mm_idx += 1

            for b in slab_bounds:
                emit_mm(0, b)

            f0 = 0
            for ci, fm in enumerate(ACT_CHUNKS):
                cs = slice(f0, f0 + fm)
                nc.scalar.activation(lx[:, cs], xt[:, cs], ACT.Ln, bias=eps_t[:])
                nc.vector.tensor_tensor(
                    sq[:, cs], lx[:, cs], lx[:, cs], AOP.mult
                )
                for s in (1, 2):
                    for j in range(fm // SLAB):
                        emit_mm(s, f0 + j * SLAB)
                f0 += fm
                if ci == 2:
                    # x accumulators are closed once the last DMA lands;
                    # drain them here so the copy+DMA overlap the Ln chain
                    nc.vector.tensor_copy(outsb[:, 0:SLAB], accs[0][:, :])
                    nc.sync.dma_start(
                        out_d[:, 0:SLAB], outsb[0:P:32, 0:SLAB]
                    )

            nc.scalar.activation(
                outsb[:, SLAB : 2 * SLAB], accs[1][:, :], ACT.Copy
            )
            nc.vector.tensor_copy(outsb[:, 2 * SLAB : 3 * SLAB], accs[2][:, :])
            nc.sync.dma_start(
                out_d[:, SLAB : 3 * SLAB], outsb[0:P:32, SLAB : 3 * SLAB]
            )

    return nc


_PROG_CACHE = {}


def _get_prog():
    key = 0
    if key not in _PROG_CACHE:
        nc = build_nc()
        fixed, _n = _split_sync_waits(nc.to_json_bytes())
        nc.to_json_bytes = lambda: fixed
        _PROG_CACHE[key] = nc
    return _PROG_CACHE[key]


def _prepare(x, idx):
    """Sort/pad edges into per-core [128, F_TOT] |x| bf16 images.

    Returns (in_maps, host state dict for the finale).
    """
    import ml_dtypes

    x = np.asarray(x, dtype=np.float32).ravel()
    idx = np.asarray(idx).ravel().astype(np.int64)
    n = x.shape[0]

    neg = (x < 0).astype(np.int64)
    key = idx * 2 + neg
    order = np.argsort(key, kind="stable")
    xs = x[order]
    ks = key[order]
    gcnt = np.bincount(key, minlength=2 * NUM_SEG)
    gstart = np.zeros(2 * NUM_SEG, dtype=np.int64)
    np.cumsum(gcnt[:-1], out=gstart[1:])
    rank = np.arange(n, dtype=np.int64) - gstart[ks]
    ok = rank < C2

    flat = np.ones(2 * NUM_SEG * C2, dtype=np.float32)
    flat[ks[ok] * C2 + rank[ok]] = np.abs(xs[ok])

    # exact host-side corrections (float64)
    spill_x = np.zeros(NUM_SEG, dtype=np.float64)
    spill_l = np.zeros(NUM_SEG, dtype=np.float64)
    spill_q = np.zeros(NUM_SEG, dtype=np.float64)
    if not ok.all():
        sp = ~ok
        seg_sp = (ks[sp] >> 1).astype(np.int64)
        xv = xs[sp].astype(np.float64)
        lv = np.log(np.abs(xv) + EPS)
        np.add.at(spill_x, seg_sp, xv)
        np.add.at(spill_l, seg_sp, lv)
        np.add.at(spill_q, seg_sp, lv * lv)

    npad = (C2 - np.minimum(gcnt, C2)).astype(np.float64)  # [1024]
    counts = np.bincount(idx, minlength=NUM_SEG).astype(np.float64)

    flat8 = flat.astype(ml_dtypes.float8_e5m2)
    padded = flat8.reshape(NUM_SEG, 2, P, TPP)
    in_maps = []
    for c in range(N_CORES):
        a = padded[c * SEG_PER_CORE : (c + 1) * SEG_PER_CORE]  # [64,2,128,T]
        img = a.transpose(2, 3, 0, 1).reshape(P, F_TOT)
        im, f0 = {}, 0
        for ci, fm in enumerate(CHUNKS):
            im[f"xs{ci}"] = np.ascontiguousarray(img[:, f0 : f0 + fm])
            f0 += fm
        in_maps.append(im)

    state = {
        "npad": npad.reshape(NUM_SEG, 2),
        "counts": counts,
        "spill": (spill_x, spill_l, spill_q),
    }
    return in_maps, state


def _fold_outputs(results):
    """Per-core [4, 1536] f32 -> [3, NUM_SEG, 2] (stream, seg, sign)."""
    sums = np.zeros((3, NUM_SEG, 2), dtype=np.float64)
    for c, res in enumerate(results):
        o = np.asarray(res["out"], dtype=np.float64)  # [4, 1536]
        for s in range(3):
            v = o[:, s * SLAB : (s + 1) * SLAB].sum(axis=0)  # [512]
            g = v.reshape(SLAB // N_GRP, N_GRP).sum(axis=0)  # [128] groups
            seg0 = c * SEG_PER_CORE
            sums[s, seg0 : seg0 + SEG_PER_CORE, :] = g.reshape(
                SEG_PER_CORE, 2
            )
    return sums


def _finale(sums, state, target_mean, target_std):
    npad = state["npad"]  # [512, 2]
    counts = state["counts"]
    spill_x, spill_l, spill_q = state["spill"]

    lpad = np.log1p(EPS)
    sx = (
        (sums[0, :, 0] - npad[:, 0])
        - (sums[0, :, 1] - npad[:, 1])
        + spill_x
    )
    sl = sums[1].sum(axis=1) - npad.sum(axis=1) * lpad + spill_l
    sq = sums[2].sum(axis=1) - npad.sum(axis=1) * lpad * lpad + spill_q

    cg = np.maximum(counts, 1.0)
    mean_w = sx / cg
    mean_log = sl / cg
    log_var = sq / cg - mean_log**2
    std_w = np.sqrt(log_var + EPS)
    tm = np.asarray(target_mean, dtype=np.float64)
    ts = np.asarray(target_std, dtype=np.float64)
    mean_loss = np.mean((mean_w - tm) ** 2)
    std_loss = np.mean((std_w - ts) ** 2)
    total = (1.0 - STD_WEIGHT) * mean_loss + STD_WEIGHT * std_loss
    return np.float32(total * STRENGTH)


def run_device(x, idx, trace=False):
    """Run the device program; returns (sums, state, res)."""
    _install_ntff_hook()
    from concourse.bass_utils import run_bass_kernel_spmd

    nc = _get_prog()
    in_maps, state = _prepare(x, idx)
    res = run_bass_kernel_spmd(
        nc, in_maps, list(range(N_CORES)), trace=trace
    )
    sums = _fold_outputs(res.results)
    return sums, state, res


def kernel(x, idx, target_mean, target_std):
    sums, state, _res = run_device(x, idx, trace=False)
    return _finale(sums, state, target_mean, target_std)


# revision 31
# speedup vs baseline: 1.1726x; 1.1726x over previous
"""Trainium2 Bass kernel for nn_MeanStdStiffRegularizer (segment reduce).

Strategy (8 NeuronCores, segment-sharded, sort-based):
  - The host shards BY SEGMENT: core c owns segments [64c, 64c+64).  Edges
    are permuted (stable sort by (segment, sign)) and laid out so that each
    (segment, sign) group occupies a fixed [128, 128] block of the per-core
    [128, 16384] fp8e5m2 image; column f = t*128 + (2*seg_local + sign).
    Only |x| is shipped (sign is encoded in the column parity), so no idx
    tensor and no abs op on device.  e5m2 keeps log-domain error ~1e-3 on
    the final loss (e4m3's subnormal floor would not).  Pads are 1.0
    (ln(1+eps) ~ 0) and group-capacity overflow (~0.3% of edges at the
    mean-sized capacity) is compensated exactly on the host in float64.
  - Device per core: Ln(|x|+eps) on ScalarE (the 1 elem/lane/cycle floor,
    chunked to pipeline behind the DMAs), L^2 on VectorE (bf16
    tensor_tensor 2x), and every segment sum on the PE: matmul with a
    ones[128,1] stationary against 512-wide moving slabs accumulated in
    PSUM.  Four PE column strips (tile_position) run concurrently; PSUM
    column j accumulates group g = j%128.  The x-stream matmuls are
    emitted first (they only depend on DMAs) so their accumulators drain
    mid-kernel, off the critical tail.
  - No collective: each core returns 3x4x[512] f32 strip partials; the
    host folds strips/replicas and does the final 512-sized math in f64.
"""

import sys
import types

import numpy as np

N_EDGES = 16777216
NUM_SEG = 512
STRENGTH = 0.01
STD_WEIGHT = 0.5
EPS = 1e-6

N_CORES = 8
P = 128
SEG_PER_CORE = NUM_SEG // N_CORES  # 64
N_GRP = 2 * SEG_PER_CORE  # 128 (seg, sign) groups per core
TPP = 128  # elems per partition per (seg, sign) group
C2 = P * TPP  # 16384 capacity per (seg, sign) group (~0.3% spill to host)
F_TOT = N_GRP * TPP  # 16384 free elems per partition
SLAB = 512
N_STRIP = 4
# DMA chunk boundaries (slab-aligned; small first chunks for fast ramp)
CHUNKS = (512, 1024, 2048, 4096, 4096, 2048, 1024, 1024, 512)
assert sum(CHUNKS) == F_TOT
# ACT/TT chunk boundaries: finer than DMA up to the last delivery-gated
# chunk (col 7680); the mid-stream chunks 4+5 merge into one Ln (their DMA
# sems fire well before the saturated Ln chain arrives, so one less
# instruction overhead on the critical path).  The tail chunks stay small:
# each tail chunk's TT + sq-matmuls drain against the remaining Ln chain,
# so a merged tail lengthens the final cascade more than it saves.
ACT_CHUNKS = (512, 1024, 2048, 4096, 6144, 1024, 1024, 512)
assert sum(ACT_CHUNKS) == F_TOT


def _install_ntff_hook():
    """Register the axon NTFF profiling hook (missing antenv.axon_hooks)."""
    if "antenv.axon_hooks" in sys.modules:
        return
    mod = types.ModuleType("antenv.axon_hooks")
    _h = [None]
    mod.set_axon_ntff_profile_hook = lambda h: _h.__setitem__(0, h)
    mod.get_axon_ntff_profile_hook = lambda: _h[0]
    sys.modules["antenv.axon_hooks"] = mod
    try:
        from trn_agent_boot.trn_boot import _ntff_profile_via_ctypes

        mod.set_axon_ntff_profile_hook(
            _ntff_profile_via_ctypes("/opt/axon/libaxon_pjrt.so")
        )
    except Exception:
        pass


_NO_SPLIT_OPCODES = {
    "CollectiveCompute",
}


def _split_sync_waits(bir_json_bytes):
    """Rewrite BIR so no TPB instruction carries more than one sync wait.

    The walrus codegen in this container supports a single sync-wait slot
    per TPB instruction ("Too many sync wait commands" otherwise).  Extra
    waits are hoisted onto EventSemaphore instructions inserted immediately
    before, on the same engine (same issue-gating semantics).
    """
    import json

    j = json.loads(bir_json_bytes)
    n_split = 0
    uid = [0]
    for f in j["functions"]:
        for b in f["blocks"]:
            out = []
            for ins in b["instructions"]:
                si = ins.get("sync_info")
                ow = (si or {}).get("on_wait") or []
                if len(ow) > 1 and ins.get("opcode") not in _NO_SPLIT_OPCODES:
                    for w in ow[:-1]:
                        uid[0] += 1
                        out.append(
                            {
                                "debug": ins.get("debug", 0),
                                "engine": ins["engine"],
                                "ins": [],
                                "name": f"{ins['name']}-wsplit{uid[0]}",
                                "opcode": "EventSemaphore",
                                "outs": [],
                                "sync_info": {"on_update": [], "on_wait": [w]},
                            }
                        )
                    si["on_wait"] = [ow[-1]]
                    n_split += 1
                out.append(ins)
            b["instructions"] = out
    return json.dumps(j).encode(), n_split


def build_nc(n_cores=N_CORES):
    """Build the per-core Bass program (SPMD: same program on every core)."""
    import concourse.bass as bass
    import concourse.tile as tile
    from concourse import mybir

    f32 = mybir.dt.float32
    bf16 = mybir.dt.bfloat16
    AOP = mybir.AluOpType
    ACT = mybir.ActivationFunctionType

    nc = bass.Bass(
        "TRN2", target_bir_lowering=False, debug=False, num_devices=n_cores
    )
    f8 = mybir.dt.float8e5
    xs_ds = [
        nc.dram_tensor(f"xs{ci}", [P, fm], f8, kind="ExternalInput")
        for ci, fm in enumerate(CHUNKS)
    ]
    out_d = nc.dram_tensor(
        "out", [N_STRIP, 3 * SLAB], f32, kind="ExternalOutput"
    )

    with tile.TileContext(nc) as tc:
        with (
            tc.tile_pool(name="const", bufs=1) as cpool,
            tc.tile_pool(name="big", bufs=1) as big,
            tc.tile_pool(name="fin", bufs=1) as fin,
            tc.tile_pool(name="acc", bufs=1, space="PSUM") as psum,
        ):
            ones = cpool.tile([P, 1], bf16)
            nc.vector.memset(ones[:], 1.0)
            ones8 = cpool.tile([P, 1], f8)
            nc.vector.memset(ones8[:], 1.0)
            eps_t = cpool.tile([P, 1], f32)
            nc.vector.memset(eps_t[:], EPS)

            # single resident tiles; DMA/compute operate on column regions
            xt = big.tile([P, F_TOT], f8, name="xt")
            lx = big.tile([P, F_TOT], bf16, name="lx")
            sq = big.tile([P, F_TOT], bf16, name="sq")

            accs = [
                psum.tile([P, SLAB], f32, tag=f"acc{s}", name=f"acc{s}")
                for s in range(3)
            ]

            # MM schedule: all x-stream matmuls first (they depend only on
            # the DMAs, so the x accumulators close early), then per chunk
            # lx-mms followed by sq-mms.  Strip of the i-th emitted matmul
            # = i % N_STRIP so consecutive PE instructions always hit
            # different column strips (max concurrency, incl. the tail).
            slab_bounds = []
            f0 = 0
            for fm in CHUNKS:
                for j in range(fm // SLAB):
                    slab_bounds.append(f0 + j * SLAB)
                f0 += fm
            n_slab_total = F_TOT // SLAB
            sched = [(0, b) for b in slab_bounds]  # x block
            f0 = 0
            for fm in ACT_CHUNKS:
                for s in (1, 2):
                    for j in range(fm // SLAB):
                        sched.append((s, f0 + j * SLAB))
                f0 += fm
            group_total = [[0] * N_STRIP for _ in range(3)]
            for i, (s, _b) in enumerate(sched):
                group_total[s][i % N_STRIP] += 1

            # input DMAs in order: chunk 0 from Scalar (earliest preamble),
            # the rest from Sync (in-order transfers keep early chunks from
            # being starved by round-robin with later big ones)
            f0 = 0
            for ci, fm in enumerate(CHUNKS):
                cs = slice(f0, f0 + fm)
                eng = nc.scalar if ci == 0 else nc.sync
                eng.dma_start(xt[:, cs], xs_ds[ci][:])
                f0 += fm

            outsb = fin.tile([P, 3 * SLAB], f32)

            nmm = [[0] * N_STRIP for _ in range(3)]
            mm_idx = 0

            def emit_mm(s, b):
                nonlocal mm_idx
                src = (xt, lx, sq)[s]
                q = mm_idx % N_STRIP
                nc.tensor.matmul(
                    accs[s][32 * q : 32 * q + 1, :],
                    ones8[:] if s == 0 else ones[:],
                    src[:, b : b + SLAB],
                    start=(nmm[s][q] == 0),
                    stop=(nmm[s][q] == group_total[s][q] - 1),
                    tile_position=(0, 32 * q),
                )
                nmm[s][q] += 1
                mm_idx += 1

            for b in slab_bounds:
                emit_mm(0, b)

            f0 = 0
            for ci, fm in enumerate(ACT_CHUNKS):
                cs = slice(f0, f0 + fm)
                nc.scalar.activation(lx[:, cs], xt[:, cs], ACT.Ln, bias=eps_t[:])
                nc.vector.tensor_tensor(
                    sq[:, cs], lx[:, cs], lx[:, cs], AOP.mult
                )
                for s in (1, 2):
                    for j in range(fm // SLAB):
                        emit_mm(s, f0 + j * SLAB)
                f0 += fm
                if ci == 2:
                    # x accumulators are closed once the last DMA lands;
                    # drain them here so the copy+DMA overlap the Ln chain
                    nc.vector.tensor_copy(outsb[:, 0:SLAB], accs[0][:, :])
                    nc.sync.dma_start(
                        out_d[:, 0:SLAB], outsb[0:P:32, 0:SLAB]
                    )

            nc.scalar.activation(
                outsb[:, SLAB : 2 * SLAB], accs[1][:, :], ACT.Copy
            )
            nc.vector.tensor_copy(outsb[:, 2 * SLAB : 3 * SLAB], accs[2][:, :])
            nc.sync.dma_start(
                out_d[:, SLAB : 3 * SLAB], outsb[0:P:32, SLAB : 3 * SLAB]
            )

    return nc


_PROG_CACHE = {}


def _get_prog():
    key = 0
    if key not in _PROG_CACHE:
        nc = build_nc()
        fixed, _n = _split_sync_waits(nc.to_json_bytes())
        nc.to_json_bytes = lambda: fixed
        _PROG_CACHE[key] = nc
    return _PROG_CACHE[key]


def _prepare(x, idx):
    """Sort/pad edges into per-core [128, F_TOT] |x| bf16 images.

    Returns (in_maps, host state dict for the finale).
    """
    import ml_dtypes

    x = np.asarray(x, dtype=np.float32).ravel()
    idx = np.asarray(idx).ravel().astype(np.int64)
    n = x.shape[0]

    neg = (x < 0).astype(np.int64)
    key = idx * 2 + neg
    order = np.argsort(key, kind="stable")
    xs = x[order]
    ks = key[order]
    gcnt = np.bincount(key, minlength=2 * NUM_SEG)
    gstart = np.zeros(2 * NUM_SEG, dtype=np.int64)
    np.cumsum(gcnt[:-1], out=gstart[1:])
    rank = np.arange(n, dtype=np.int64) - gstart[ks]
    ok = rank < C2

    flat = np.ones(2 * NUM_SEG * C2, dtype=np.float32)
    flat[ks[ok] * C2 + rank[ok]] = np.abs(xs[ok])

    # exact host-side corrections (float64)
    spill_x = np.zeros(NUM_SEG, dtype=np.float64)
    spill_l = np.zeros(NUM_SEG, dtype=np.float64)
    spill_q = np.zeros(NUM_SEG, dtype=np.float64)
    if not ok.all():
        sp = ~ok
        seg_sp = (ks[sp] >> 1).astype(np.int64)
        xv = xs[sp].astype(np.float64)
        lv = np.log(np.abs(xv) + EPS)
        np.add.at(spill_x, seg_sp, xv)
        np.add.at(spill_l, seg_sp, lv)
        np.add.at(spill_q, seg_sp, lv * lv)

    npad = (C2 - np.minimum(gcnt, C2)).astype(np.float64)  # [1024]
    counts = np.bincount(idx, minlength=NUM_SEG).astype(np.float64)

    flat8 = flat.astype(ml_dtypes.float8_e5m2)
    padded = flat8.reshape(NUM_SEG, 2, P, TPP)
    in_maps = []
    for c in range(N_CORES):
        a = padded[c * SEG_PER_CORE : (c + 1) * SEG_PER_CORE]  # [64,2,128,T]
        img = a.transpose(2, 3, 0, 1).reshape(P, F_TOT)
        im, f0 = {}, 0
        for ci, fm in enumerate(CHUNKS):
            im[f"xs{ci}"] = np.ascontiguousarray(img[:, f0 : f0 + fm])
            f0 += fm
        in_maps.append(im)

    state = {
        "npad": npad.reshape(NUM_SEG, 2),
        "counts": counts,
        "spill": (spill_x, spill_l, spill_q),
    }
    return in_maps, state


def _fold_outputs(results):
    """Per-core [4, 1536] f32 -> [3, NUM_SEG, 2] (stream, seg, sign)."""
    sums = np.zeros((3, NUM_SEG, 2), dtype=np.float64)
    for c, res in enumerate(results):
        o = np.asarray(res["out"], dtype=np.float64)  # [4, 1536]
        for s in range(3):
            v = o[:, s * SLAB : (s + 1) * SLAB].sum(axis=0)  # [512]
            g = v.reshape(SLAB // N_GRP, N_GRP).sum(axis=0)  # [128] groups
            seg0 = c * SEG_PER_CORE
            sums[s, seg0 : seg0 + SEG_PER_CORE, :] = g.reshape(
                SEG_PER_CORE, 2
            )
    return sums


def _finale(sums, state, target_mean, target_std):
    npad = state["npad"]  # [512, 2]
    counts = state["counts"]
    spill_x, spill_l, spill_q = state["spill"]

    lpad = np.log1p(EPS)
    sx = (
        (sums[0, :, 0] - npad[:, 0])
        - (sums[0, :, 1] - npad[:, 1])
        + spill_x
    )
    sl = sums[1].sum(axis=1) - npad.sum(axis=1) * lpad + spill_l
    sq = sums[2].sum(axis=1) - npad.sum(axis=1) * lpad * lpad + spill_q

    cg = np.maximum(counts, 1.0)
    mean_w = sx / cg
    mean_log = sl / cg
    log_var = sq / cg - mean_log**2
    std_w = np.sqrt(log_var + EPS)
    tm = np.asarray(target_mean, dtype=np.float64)
    ts = np.asarray(target_std, dtype=np.float64)
    mean_loss = np.mean((mean_w - tm) ** 2)
    std_loss = np.mean((std_w - ts) ** 2)
    total = (1.0 - STD_WEIGHT) * mean_loss + STD_WEIGHT * std_loss
    return np.float32(total * STRENGTH)


def run_device(x, idx, trace=False):
    """Run the device program; returns (sums, state, res)."""
    _install_ntff_hook()
    from concourse.bass_utils import run_bass_kernel_spmd

    nc = _get_prog()
    in_maps, state = _prepare(x, idx)
    res = run_bass_kernel_spmd(
        nc, in_maps, list(range(N_CORES)), trace=trace
    )
    sums = _fold_outputs(res.results)
    return sums, state, res


def kernel(x, idx, target_mean, target_std):
    sums, state, _res = run_device(x, idx, trace=False)
    return _finale(sums, state, target_mean, target_std)


# revision 32
# speedup vs baseline: 1.2093x; 1.0313x over previous
"""Trainium2 Bass kernel for nn_MeanStdStiffRegularizer (segment reduce).

Strategy (8 NeuronCores, segment-sharded, sort-based):
  - The host shards BY SEGMENT: core c owns segments [64c, 64c+64).  Edges
    are permuted (stable sort by (segment, sign)) and laid out so that each
    (segment, sign) group occupies a fixed [128, 128] block of the per-core
    [128, 16384] fp8e5m2 image; column f = t*128 + (2*seg_local + sign).
    Only |x| is shipped (sign is encoded in the column parity), so no idx
    tensor and no abs op on device.  e5m2 keeps log-domain error ~1e-3 on
    the final loss (e4m3's subnormal floor would not).  Pads are 1.0
    (ln(1+eps) ~ 0) and group-capacity overflow (~0.3% of edges at the
    mean-sized capacity) is compensated exactly on the host in float64.
  - Device per core: Ln(|x|+eps) on ScalarE (the 1 elem/lane/cycle floor,
    chunked to pipeline behind the DMAs), L^2 on VectorE (bf16
    tensor_tensor 2x), and every segment sum on the PE: matmul with a
    ones[128,1] stationary against 512-wide moving slabs accumulated in
    PSUM.  Four PE column strips (tile_position) run concurrently; PSUM
    column j accumulates group g = j%128.  The x-stream matmuls are
    emitted first (they only depend on DMAs) so their accumulators drain
    mid-kernel, off the critical tail.
  - No collective: each core returns 3x4x[512] f32 strip partials; the
    host folds strips/replicas and does the final 512-sized math in f64.
"""

import sys
import types

import numpy as np

N_EDGES = 16777216
NUM_SEG = 512
STRENGTH = 0.01
STD_WEIGHT = 0.5
EPS = 1e-6

N_CORES = 8
P = 128
SEG_PER_CORE = NUM_SEG // N_CORES  # 64
N_GRP = 2 * SEG_PER_CORE  # 128 (seg, sign) groups per core
TPP = 128  # elems per partition per (seg, sign) group
C2 = P * TPP  # 16384 capacity per (seg, sign) group (~0.3% spill to host)
F_TOT = N_GRP * TPP  # 16384 free elems per partition
SLAB = 512
N_STRIP = 4
# DMA chunk boundaries (slab-aligned; small first chunks for fast ramp)
CHUNKS = (512, 1024, 2048, 4096, 4096, 2048, 1024, 1024, 512)
assert sum(CHUNKS) == F_TOT
# ACT/TT chunks match the DMA chunks.  Merging Ln instructions was tried
# (saves 293ns of ACT overhead per merge) but loses more: a merged Ln
# delays TT/matmul availability for all its columns until it ends, which
# lengthens the tail cascade (measured +2us for a 6144-col merge).
ACT_CHUNKS = CHUNKS


def _install_ntff_hook():
    """Register the axon NTFF profiling hook (missing antenv.axon_hooks)."""
    if "antenv.axon_hooks" in sys.modules:
        return
    mod = types.ModuleType("antenv.axon_hooks")
    _h = [None]
    mod.set_axon_ntff_profile_hook = lambda h: _h.__setitem__(0, h)
    mod.get_axon_ntff_profile_hook = lambda: _h[0]
    sys.modules["antenv.axon_hooks"] = mod
    try:
        from trn_agent_boot.trn_boot import _ntff_profile_via_ctypes

        mod.set_axon_ntff_profile_hook(
            _ntff_profile_via_ctypes("/opt/axon/libaxon_pjrt.so")
        )
    except Exception:
        pass


_NO_SPLIT_OPCODES = {
    "CollectiveCompute",
}


def _split_sync_waits(bir_json_bytes):
    """Rewrite BIR so no TPB instruction carries more than one sync wait.

    The walrus codegen in this container supports a single sync-wait slot
    per TPB instruction ("Too many sync wait commands" otherwise).  Extra
    waits are hoisted onto EventSemaphore instructions inserted immediately
    before, on the same engine (same issue-gating semantics).
    """
    import json

    j = json.loads(bir_json_bytes)
    n_split = 0
    uid = [0]
    for f in j["functions"]:
        for b in f["blocks"]:
            out = []
            for ins in b["instructions"]:
                si = ins.get("sync_info")
                ow = (si or {}).get("on_wait") or []
                if len(ow) > 1 and ins.get("opcode") not in _NO_SPLIT_OPCODES:
                    for w in ow[:-1]:
                        uid[0] += 1
                        out.append(
                            {
                                "debug": ins.get("debug", 0),
                                "engine": ins["engine"],
                                "ins": [],
                                "name": f"{ins['name']}-wsplit{uid[0]}",
                                "opcode": "EventSemaphore",
                                "outs": [],
                                "sync_info": {"on_update": [], "on_wait": [w]},
                            }
                        )
                    si["on_wait"] = [ow[-1]]
                    n_split += 1
                out.append(ins)
            b["instructions"] = out
    return json.dumps(j).encode(), n_split


def build_nc(n_cores=N_CORES):
    """Build the per-core Bass program (SPMD: same program on every core)."""
    import concourse.bass as bass
    import concourse.tile as tile
    from concourse import mybir

    f32 = mybir.dt.float32
    bf16 = mybir.dt.bfloat16
    AOP = mybir.AluOpType
    ACT = mybir.ActivationFunctionType

    nc = bass.Bass(
        "TRN2", target_bir_lowering=False, debug=False, num_devices=n_cores
    )
    f8 = mybir.dt.float8e5
    xs_ds = [
        nc.dram_tensor(f"xs{ci}", [P, fm], f8, kind="ExternalInput")
        for ci, fm in enumerate(CHUNKS)
    ]
    out_d = nc.dram_tensor(
        "out", [N_STRIP, 3 * SLAB], f32, kind="ExternalOutput"
    )

    with tile.TileContext(nc) as tc:
        with (
            tc.tile_pool(name="const", bufs=1) as cpool,
            tc.tile_pool(name="big", bufs=1) as big,
            tc.tile_pool(name="fin", bufs=1) as fin,
            tc.tile_pool(name="acc", bufs=1, space="PSUM") as psum,
        ):
            ones = cpool.tile([P, 1], bf16)
            nc.vector.memset(ones[:], 1.0)
            ones8 = cpool.tile([P, 1], f8)
            nc.vector.memset(ones8[:], 1.0)
            eps_t = cpool.tile([P, 1], f32)
            nc.vector.memset(eps_t[:], EPS)

            # single resident tiles; DMA/compute operate on column regions
            xt = big.tile([P, F_TOT], f8, name="xt")
            lx = big.tile([P, F_TOT], bf16, name="lx")
            sq = big.tile([P, F_TOT], bf16, name="sq")

            accs = [
                psum.tile([P, SLAB], f32, tag=f"acc{s}", name=f"acc{s}")
                for s in range(3)
            ]

            # MM schedule: all x-stream matmuls first (they depend only on
            # the DMAs, so the x accumulators close early), then per chunk
            # lx-mms followed by sq-mms.  Strip of the i-th emitted matmul
            # = i % N_STRIP so consecutive PE instructions always hit
            # different column strips (max concurrency, incl. the tail).
            slab_bounds = []
            f0 = 0
            for fm in CHUNKS:
                for j in range(fm // SLAB):
                    slab_bounds.append(f0 + j * SLAB)
                f0 += fm
            n_slab_total = F_TOT // SLAB
            sched = [(0, b) for b in slab_bounds]  # x block
            f0 = 0
            for fm in ACT_CHUNKS:
                for s in (1, 2):
                    for j in range(fm // SLAB):
                        sched.append((s, f0 + j * SLAB))
                f0 += fm
            group_total = [[0] * N_STRIP for _ in range(3)]
            for i, (s, _b) in enumerate(sched):
                group_total[s][i % N_STRIP] += 1

            # input DMAs in order: chunk 0 from Scalar (earliest preamble),
            # the rest from Sync (in-order transfers keep early chunks from
            # being starved by round-robin with later big ones)
            f0 = 0
            for ci, fm in enumerate(CHUNKS):
                cs = slice(f0, f0 + fm)
                eng = nc.scalar if ci == 0 else nc.sync
                eng.dma_start(xt[:, cs], xs_ds[ci][:])
                f0 += fm

            outsb = fin.tile([P, 3 * SLAB], f32)

            nmm = [[0] * N_STRIP for _ in range(3)]
            mm_idx = 0

            def emit_mm(s, b):
                nonlocal mm_idx
                src = (xt, lx, sq)[s]
                q = mm_idx % N_STRIP
                nc.tensor.matmul(
                    accs[s][32 * q : 32 * q + 1, :],
                    ones8[:] if s == 0 else ones[:],
                    src[:, b : b + SLAB],
                    start=(nmm[s][q] == 0),
                    stop=(nmm[s][q] == group_total[s][q] - 1),
                    tile_position=(0, 32 * q),
                )
                nmm[s][q] += 1
                mm_idx += 1

            for b in slab_bounds:
                emit_mm(0, b)

            f0 = 0
            for ci, fm in enumerate(ACT_CHUNKS):
                cs = slice(f0, f0 + fm)
                nc.scalar.activation(lx[:, cs], xt[:, cs], ACT.Ln, bias=eps_t[:])
                nc.vector.tensor_tensor(
                    sq[:, cs], lx[:, cs], lx[:, cs], AOP.mult
                )
                for s in (1, 2):
                    for j in range(fm // SLAB):
                        emit_mm(s, f0 + j * SLAB)
                f0 += fm
                if ci == 2:
                    # x accumulators are closed once the last DMA lands;
                    # drain them here so the copy+DMA overlap the Ln chain
                    nc.vector.tensor_copy(outsb[:, 0:SLAB], accs[0][:, :])
                    nc.sync.dma_start(
                        out_d[:, 0:SLAB], outsb[0:P:32, 0:SLAB]
                    )

            nc.scalar.activation(
                outsb[:, SLAB : 2 * SLAB], accs[1][:, :], ACT.Copy
            )
            nc.vector.tensor_copy(outsb[:, 2 * SLAB : 3 * SLAB], accs[2][:, :])
            nc.sync.dma_start(
                out_d[:, SLAB : 3 * SLAB], outsb[0:P:32, SLAB : 3 * SLAB]
            )

    return nc


_PROG_CACHE = {}


def _get_prog():
    key = 0
    if key not in _PROG_CACHE:
        nc = build_nc()
        fixed, _n = _split_sync_waits(nc.to_json_bytes())
        nc.to_json_bytes = lambda: fixed
        _PROG_CACHE[key] = nc
    return _PROG_CACHE[key]


def _prepare(x, idx):
    """Sort/pad edges into per-core [128, F_TOT] |x| bf16 images.

    Returns (in_maps, host state dict for the finale).
    """
    import ml_dtypes

    x = np.asarray(x, dtype=np.float32).ravel()
    idx = np.asarray(idx).ravel().astype(np.int64)
    n = x.shape[0]

    neg = (x < 0).astype(np.int64)
    key = idx * 2 + neg
    order = np.argsort(key, kind="stable")
    xs = x[order]
    ks = key[order]
    gcnt = np.bincount(key, minlength=2 * NUM_SEG)
    gstart = np.zeros(2 * NUM_SEG, dtype=np.int64)
    np.cumsum(gcnt[:-1], out=gstart[1:])
    rank = np.arange(n, dtype=np.int64) - gstart[ks]
    ok = rank < C2

    flat = np.ones(2 * NUM_SEG * C2, dtype=np.float32)
    flat[ks[ok] * C2 + rank[ok]] = np.abs(xs[ok])

    # exact host-side corrections (float64)
    spill_x = np.zeros(NUM_SEG, dtype=np.float64)
    spill_l = np.zeros(NUM_SEG, dtype=np.float64)
    spill_q = np.zeros(NUM_SEG, dtype=np.float64)
    if not ok.all():
        sp = ~ok
        seg_sp = (ks[sp] >> 1).astype(np.int64)
        xv = xs[sp].astype(np.float64)
        lv = np.log(np.abs(xv) + EPS)
        np.add.at(spill_x, seg_sp, xv)
        np.add.at(spill_l, seg_sp, lv)
        np.add.at(spill_q, seg_sp, lv * lv)

    npad = (C2 - np.minimum(gcnt, C2)).astype(np.float64)  # [1024]
    counts = np.bincount(idx, minlength=NUM_SEG).astype(np.float64)

    flat8 = flat.astype(ml_dtypes.float8_e5m2)
    padded = flat8.reshape(NUM_SEG, 2, P, TPP)
    in_maps = []
    for c in range(N_CORES):
        a = padded[c * SEG_PER_CORE : (c + 1) * SEG_PER_CORE]  # [64,2,128,T]
        img = a.transpose(2, 3, 0, 1).reshape(P, F_TOT)
        im, f0 = {}, 0
        for ci, fm in enumerate(CHUNKS):
            im[f"xs{ci}"] = np.ascontiguousarray(img[:, f0 : f0 + fm])
            f0 += fm
        in_maps.append(im)

    state = {
        "npad": npad.reshape(NUM_SEG, 2),
        "counts": counts,
        "spill": (spill_x, spill_l, spill_q),
    }
    return in_maps, state


def _fold_outputs(results):
    """Per-core [4, 1536] f32 -> [3, NUM_SEG, 2] (stream, seg, sign)."""
    sums = np.zeros((3, NUM_SEG, 2), dtype=np.float64)
    for c, res in enumerate(results):
        o = np.asarray(res["out"], dtype=np.float64)  # [4, 1536]
        for s in range(3):
            v = o[:, s * SLAB : (s + 1) * SLAB].sum(axis=0)  # [512]
            g = v.reshape(SLAB // N_GRP, N_GRP).sum(axis=0)  # [128] groups
            seg0 = c * SEG_PER_CORE
            sums[s, seg0 : seg0 + SEG_PER_CORE, :] = g.reshape(
                SEG_PER_CORE, 2
            )
    return sums


def _finale(sums, state, target_mean, target_std):
    npad = state["npad"]  # [512, 2]
    counts = state["counts"]
    spill_x, spill_l, spill_q = state["spill"]

    lpad = np.log1p(EPS)
    sx = (
        (sums[0, :, 0] - npad[:, 0])
        - (sums[0, :, 1] - npad[:, 1])
        + spill_x
    )
    sl = sums[1].sum(axis=1) - npad.sum(axis=1) * lpad + spill_l
    sq = sums[2].sum(axis=1) - npad.sum(axis=1) * lpad * lpad + spill_q

    cg = np.maximum(counts, 1.0)
    mean_w = sx / cg
    mean_log = sl / cg
    log_var = sq / cg - mean_log**2
    std_w = np.sqrt(log_var + EPS)
    tm = np.asarray(target_mean, dtype=np.float64)
    ts = np.asarray(target_std, dtype=np.float64)
    mean_loss = np.mean((mean_w - tm) ** 2)
    std_loss = np.mean((std_w - ts) ** 2)
    total = (1.0 - STD_WEIGHT) * mean_loss + STD_WEIGHT * std_loss
    return np.float32(total * STRENGTH)


def run_device(x, idx, trace=False):
    """Run the device program; returns (sums, state, res)."""
    _install_ntff_hook()
    from concourse.bass_utils import run_bass_kernel_spmd

    nc = _get_prog()
    in_maps, state = _prepare(x, idx)
    res = run_bass_kernel_spmd(
        nc, in_maps, list(range(N_CORES)), trace=trace
    )
    sums = _fold_outputs(res.results)
    return sums, state, res


def kernel(x, idx, target_mean, target_std):
    sums, state, _res = run_device(x, idx, trace=False)
    return _finale(sums, state, target_mean, target_std)
